# revision 29
# baseline (speedup 1.0000x reference)
"""KAConv (rational-function conv) Trainium2 Bass kernel, 8-core SPMD.

Math per output (b,f,h,w):
  out = sum_{c,p} P_fcp(x_win) / (1 + |Q_fcp(x_win)|)
with P = deg-5 poly (6 coeffs), Q = powers 1..4 (4 coeffs), win = 3x3 offsets.

Strategy (all shapes hardcoded for B=4,C=16,F=16,H=W=64,K=3):
- Shard spatial: core k handles batch k//2, H-rows 32*(k%2) .. +32  (2048 pts).
- On device, build power tensors PW [48, 2244] = rows (k*8 + c_local) holding
  x^k for 8 channels over the 34x66 zero-padded slice; two tensors for the
  two channel octets. x ships as fp16 (half the bytes; rel err ~4e-3, well
  inside the 2e-2 gate) and is upconverted on device.
- Coefficients (+ fold selector) ship compact and SHARDED: 11 rows/core of
  an [88, 288] fp16 block, AllGathered on device, upconverted to f32 on
  SBUF, then scattered into the block-diagonal lhsT layout ([48, 18*128])
  via 80 single-partition DMAs after a memset. This replaces 8.8 MB of
  host->device traffic (93%-zero block-diag tensors, replicated x8) with
  ~0.62 MB total. Total wire error (x fp16 + coef fp16 + out fp16) is
  5.6e-3 vs the 2e-2 gate.
- P and Q for 8 channels x 16 filters at once via one K=48, M=128, N=512
  block-diagonal matmul on TensorE per (octet, kernel-offset, 512-pt chunk).
- Consumers (full 128-lane ops): |q| -> Abs (ACT), ln(1+|q|) (ACT, bias=1),
  r=exp(-l) (ACT), t = P*r (DVE TT), then sum over channels via a selector
  matmul E.T @ t accumulated into PSUM across all 72 units.
- The per-core fp16 result is AllGathered on device into a replicated
  [128, 2048] output, so the host fetch is one round-trip from one device.
- Dispatch: the shard_map executable is built and AOT-compiled once and
  cached (run_bass_kernel_spmd re-traces every call, ~185 ms). The donated
  output buffer is recycled from the previous call's device-resident
  result, so no zero-buffer upload per call. First kernel() call runs a
  few warmups so subsequent calls hit the steady-state fast path (~68 ms,
  dominated by the axon-tunnel execute round-trip).
"""

import numpy as np

import jax
from jax.sharding import Mesh, PartitionSpec
from jax.experimental.shard_map import shard_map

import concourse.bass as bass
import concourse.bacc as bacc
import concourse.tile as tile
import concourse.mybir as mybir
from concourse.bass2jax import (
    _bass_exec_p,
    partition_id_tensor,
    install_neuronx_cc_hook,
)

F32 = mybir.dt.float32
F16 = mybir.dt.float16
AF = mybir.ActivationFunctionType

B, C, F, H, W = 4, 16, 16, 64, 64
PH, PW_ = 34, 66          # padded slice dims per core (32+2 rows, 64+2 cols)
NPIX = PH * PW_           # 2244
ROWS, CHUNK = 32, 512     # output rows per core, free-dim chunk (8 rows x 64)
NCH = 4                   # chunks per core (4 x 512 = 2048 pts)
DEG_P, DEG_Q, KK = 6, 4, 9
NCORES = 8

_cache = {}


def _build_program():
    nc = bacc.Bacc("TRN2", target_bir_lowering=False, debug=False, num_devices=NCORES)

    xh = nc.dram_tensor("xh", [C, NPIX], F16, kind="ExternalInput").ap()
    # compact coefficients arrive fp16, sharded 11 rows/core, AllGathered on
    # device: rows 0..47 A, 48..79 Bc, 80..87 the fold selector (16x16 per row)
    cshard = nc.dram_tensor("cshard", [11, 2 * KK * 16], F16, kind="ExternalInput").ap()
    cstage = nc.dram_tensor("cstage", [11, 2 * KK * 16], F16, kind="Internal").ap()
    cfull = nc.dram_tensor(
        "cfull", [88, 2 * KK * 16], F16, kind="Internal", addr_space="Shared"
    ).ap()
    # per-core result is AllGathered on device so the host fetches the full
    # output from a single device (1 fetch round-trip instead of 8)
    ostage = nc.dram_tensor("ostage", [16, ROWS * 64], F16, kind="Internal").ap()
    ofull = nc.dram_tensor(
        "ofull", [128, ROWS * 64], F16, kind="Internal", addr_space="Shared"
    ).ap()
    out = nc.dram_tensor("out", [128, ROWS * 64], F16, kind="ExternalOutput").ap()

    with tile.TileContext(nc) as tc:
        with (
            tc.tile_pool(name="persist", bufs=1) as pp_persist,
            tc.tile_pool(name="work", bufs=4) as pw_work,
            tc.tile_pool(name="psum", bufs=2, space=bass.MemorySpace.PSUM) as pp_psum,
            tc.tile_pool(name="psacc", bufs=1, space=bass.MemorySpace.PSUM) as pp_acc,
        ):
            # ---- coefficient AllGather kicked off first (overlaps x work) ----
            nc.sync.dma_start(cstage[:], cshard[:])
            nc.gpsimd.collective_compute(
                "AllGather",
                mybir.AluOpType.bypass,
                replica_groups=[list(range(NCORES))],
                ins=[cstage],
                outs=[cfull],
            )

            # ---- x: fp16 in, upconvert to f32 rows 8..16 of both PW tensors ----
            xhs = pp_persist.tile([C, NPIX], F16, tag="xhs")
            nc.sync.dma_start(xhs[:], xh[:])

            x1 = pp_persist.tile([C, NPIX], F32, tag="x1")
            nc.scalar.activation(x1[:], xhs[:], AF.Copy)
            x2 = pp_persist.tile([C, NPIX], F32, tag="x2")
            nc.vector.tensor_mul(x2[:], x1[:], x1[:])
            x3 = pp_persist.tile([C, NPIX], F32, tag="x3")
            nc.vector.tensor_mul(x3[:], x2[:], x1[:])
            x4 = pp_persist.tile([C, NPIX], F32, tag="x4")
            nc.vector.tensor_mul(x4[:], x2[:], x2[:])
            x5 = pp_persist.tile([C, NPIX], F32, tag="x5")
            nc.vector.tensor_mul(x5[:], x2[:], x3[:])

            # PW tensors: rows 8k + cl hold x_{oct*8+cl}^k
            pwa = pp_persist.tile([48, NPIX], F32, tag="pwa")
            pwb = pp_persist.tile([48, NPIX], F32, tag="pwb")
            for oct_, pwt in ((0, pwa), (1, pwb)):
                nc.vector.memset(pwt[0:8, :], 1.0)
                for k, xk in ((1, x1), (2, x2), (3, x3), (4, x4), (5, x5)):
                    nc.sync.dma_start(
                        pwt[8 * k : 8 * k + 8, :], xk[8 * oct_ : 8 * oct_ + 8, :]
                    )

            # ---- block-diag lhsT tiles scattered from compact coef ----
            # gathered fp16 coef is upconverted once on SBUF, then scattered
            cf16 = pp_persist.tile([88, 2 * KK * 16], F16, tag="cf16")
            nc.sync.dma_start(cf16[:], cfull[:])
            cf32 = pp_persist.tile([88, 2 * KK * 16], F32, tag="cf32")
            nc.scalar.activation(cf32[:], cf16[:], AF.Copy)

            cps = pp_persist.tile([48, 2 * KK * 128], F32, tag="cps")
            cqs = pp_persist.tile([48, 2 * KK * 128], F32, tag="cqs")
            nc.vector.memset(cps[:], 0.0)
            nc.vector.memset(cqs[:], 0.0)
            # one DMA per compact row: row r of A lands in cps partition r at
            # column offset 16*(r%8) of each 128-wide block; B rows shift down
            # one power-octet (8 partitions). Single-partition APs keep the
            # race tracker exact (partition-strided writes are over-approximated).
            cview = cps[:].rearrange("r (u w) -> r u w", w=128)
            qview = cqs[:].rearrange("r (u w) -> r u w", w=128)
            csrc = cf32[:].rearrange("r (u v) -> r u v", v=16)
            for r in range(48):
                cl = r % 8
                nc.sync.dma_start(
                    cview[r : r + 1, :, 16 * cl : 16 * cl + 16], csrc[r : r + 1]
                )
            for r in range(32):
                cl = r % 8
                nc.sync.dma_start(
                    qview[r + 8 : r + 9, :, 16 * cl : 16 * cl + 16],
                    csrc[48 + r : 49 + r],
                )

            ef = pp_persist.tile([128, 16], F32, tag="ef")
            nc.sync.dma_start(
                ef[:],
                cf32[80:88, 0 : 16 * 16].rearrange("e (q v) -> e q v", v=16),
            )

            acc = pp_acc.tile([16, NCH * CHUNK], F32, tag="acc")
            osb = pp_persist.tile([16, NCH * CHUNK], F16, tag="osb")

            # ---- main loop ----
            # fold-MM for unit u is emitted after unit u+1's P/Q matmuls so
            # the in-order PE queue never stalls behind unit u's ACT/DVE chain.
            pending = None  # (tt, ch, first)
            folds_done = [0] * NCH

            def emit_fold(pend):
                tt_, ch_, first_ = pend
                folds_done[ch_] += 1
                nc.tensor.matmul(
                    acc[:, ch_ * CHUNK : (ch_ + 1) * CHUNK],
                    ef[:],
                    tt_[:],
                    start=first_,
                    stop=folds_done[ch_] == 2 * KK,
                    skip_group_check=True,
                )

            for oct_ in range(2):
                pwt = pwa if oct_ == 0 else pwb
                pw3 = pwt[:].rearrange("p (h w) -> p h w", w=PW_)
                for p in range(KK):
                    di, dj = p // 3, p % 3
                    lhsP = cps[:, (oct_ * KK + p) * 128 : (oct_ * KK + p) * 128 + 128]
                    lhsQ = cqs[:, (oct_ * KK + p) * 128 : (oct_ * KK + p) * 128 + 128]
                    for ch in range(NCH):
                        r0 = ch * 8 + di
                        rhs = pw3[:, r0 : r0 + 8, dj : dj + 64]
                        pp = pp_psum.tile([128, CHUNK], F32, tag="pp")
                        nc.tensor.matmul(pp[:], lhsP, rhs, start=True, stop=True)
                        qq = pp_psum.tile([128, CHUNK], F32, tag="qq")
                        nc.tensor.matmul(qq[:], lhsQ, rhs, start=True, stop=True)
                        if pending is not None:
                            emit_fold(pending)

                        dd = pw_work.tile([128, CHUNK], F32, tag="dd")
                        nc.scalar.activation(dd[:], qq[:], AF.Abs)
                        ll = pw_work.tile([128, CHUNK], F32, tag="ll")
                        nc.scalar.activation(ll[:], dd[:], AF.Ln, bias=1.0)
                        rr = pw_work.tile([128, CHUNK], F32, tag="rr")
                        nc.scalar.activation(rr[:], ll[:], AF.Exp, scale=-1.0)
                        tt = pw_work.tile([128, CHUNK], F32, tag="tt")
                        nc.vector.tensor_mul(tt[:], pp[:], rr[:])
                        pending = (tt, ch, folds_done[ch] == 0)
            emit_fold(pending)

            nc.scalar.activation(osb[:], acc[:], AF.Copy)
            nc.sync.dma_start(ostage[:], osb[:])
            nc.gpsimd.collective_compute(
                "AllGather",
                mybir.AluOpType.bypass,
                replica_groups=[list(range(NCORES))],
                ins=[ostage],
                outs=[ofull],
            )
            nc.sync.dma_start(out[:], ofull[:])

    nc.compile()
    return nc


def _make_dispatch(nc):
    """Build the cached jitted shard_map executable (bass_exec custom call)."""
    install_neuronx_cc_hook()
    partition_name = nc.partition_id_tensor.name if nc.partition_id_tensor else None
    in_names, out_names, out_avals = [], [], []
    for alloc in nc.m.functions[0].allocations:
        if not isinstance(alloc, mybir.MemoryLocationSet):
            continue
        name = alloc.memorylocations[0].name
        if alloc.kind == "ExternalInput":
            if name != partition_name:
                in_names.append(name)
        elif alloc.kind == "ExternalOutput":
            out_names.append(name)
            out_avals.append(
                jax.core.ShapedArray(
                    tuple(alloc.tensor_shape), mybir.dt.np(alloc.dtype)
                )
            )
    n_params = len(in_names)
    in_names_full = in_names + out_names
    if partition_name is not None:
        in_names_full.append(partition_name)

    def _body(*args):
        operands = list(args)
        if partition_name is not None:
            operands.append(partition_id_tensor())
        outs = _bass_exec_p.bind(
            *operands,
            out_avals=tuple(out_avals),
            in_names=tuple(in_names_full),
            out_names=tuple(out_names),
            lowering_input_output_aliases=(),
            sim_require_finite=True,
            sim_require_nnan=True,
            nc=nc,
        )
        return tuple(outs)

    donate = tuple(range(n_params, n_params + len(out_names)))
    devices = jax.devices()[:NCORES]
    mesh = Mesh(np.asarray(devices), ("core",))
    # inputs row-sharded; the (donated) output is replicated — every core
    # holds the full AllGathered result, so the host fetch reads one shard
    fn = jax.jit(
        shard_map(
            _body,
            mesh=mesh,
            in_specs=(PartitionSpec("core"),) * n_params
            + (PartitionSpec(),) * len(out_names),
            out_specs=(PartitionSpec(),) * len(out_names),
            check_rep=False,
        ),
        donate_argnums=donate,
        keep_unused=True,
    )
    return fn, in_names, out_names


def _prep_inputs(x, A, Bc):
    """Host-side marshalling: fp16 padded slices + compact coefficient tiles."""
    # core k = (batch k//2, half k%2) holds padded rows 32*half..+34. The
    # zero borders of the staging buffer are static, so it is allocated once
    # and only the data interior is rewritten per call.
    xh4 = _cache.get("xh_buf")
    if xh4 is None:
        xh4 = _cache["xh_buf"] = np.zeros((NCORES, C, PH, PW_), np.float16)
    x16 = x.astype(np.float16)
    for k in range(NCORES):
        bk, half = k // 2, k % 2
        if half == 0:
            xh4[k, :, 1:34, 1:65] = x16[bk, :, 0:33, :]
        else:
            xh4[k, :, 0:33, 1:65] = x16[bk, :, 31:64, :]
    xh = xh4.reshape(NCORES * C, NPIX)

    # compact coefficients: rows 0..47 = A (8k+cl), rows 48..79 = Bc (8j+cl);
    # cols = (oct*9 + p)*16 + f
    ca = (
        A.transpose(3, 1, 2, 0)            # [k, c, p, f]
        .reshape(DEG_P, 2, 8, KK, 16)      # [k, oct, cl, p, f]
        .transpose(0, 2, 1, 3, 4)          # [k, cl, oct, p, f]
        .reshape(48, 2 * KK * 16)
    )
    cb = (
        Bc.transpose(3, 1, 2, 0)
        .reshape(DEG_Q, 2, 8, KK, 16)
        .transpose(0, 2, 1, 3, 4)
        .reshape(32, 2 * KK * 16)
    )
    ef = np.zeros((128, 16), np.float32)
    ef[np.arange(128), np.arange(128) % 16] = 1.0
    efpack = np.zeros((8, 2 * KK * 16), np.float32)
    efpack[:, : 16 * 16] = ef.reshape(8, 256)

    # global [88, 288] fp16 (0/1 selector rows are exact in fp16):
    # jax row-shards it 11 rows/core; device AllGather rebuilds
    cshard_all = np.concatenate([ca, cb, efpack], axis=0).astype(np.float16)
    return xh, cshard_all


def _run(xh, cshard_all):
    fn = _cache.get("fn_compiled") or _cache["fn"]
    prev_out = _cache.get("prev_out")
    if prev_out is None:
        prev_out = np.zeros((NCORES * 16, ROWS * 64), np.float16)  # [128, 2048]
    try:
        (out,) = fn(xh, cshard_all, prev_out)
    except Exception:
        if "fn_compiled" not in _cache:
            raise
        # AOT path rejected the inputs; fall back to the traced jit
        _cache.pop("fn_compiled")
        (out,) = _cache["fn"](xh, cshard_all, prev_out)
    try:
        # queue the D2H copy behind the execute now rather than when
        # np.asarray blocks — starts the fetch one round-trip earlier
        out.copy_to_host_async()
    except Exception:
        pass
    res = np.asarray(out)
    _cache["prev_out"] = out
    return res


def kernel(x, A, Bc):
    x = np.asarray(x, np.float32)
    A = np.asarray(A, np.float32)
    Bc = np.asarray(Bc, np.float32)
    xh, cshard_all = _prep_inputs(x, A, Bc)
    if "fn" not in _cache:
        nc = _build_program()
        fn, in_names, out_names = _make_dispatch(nc)
        assert in_names == ["xh", "cshard"] and out_names == ["out"]
        _cache["nc"] = nc
        _cache["fn"] = fn
        # AOT-compile to skip per-call jit dispatch bookkeeping (~3 ms)
        try:
            _cache["fn_compiled"] = fn.lower(
                xh,
                cshard_all,
                np.zeros((NCORES * 16, ROWS * 64), np.float16),
            ).compile()
        except Exception:
            pass
        # warm the axon/PJRT path (through the same executable later calls
        # use) so they hit steady state
        for _ in range(5):
            res = _run(xh, cshard_all)
    res = _run(xh, cshard_all)
    # res: [8*16, 2048] fp16, core k = (batch k//2, rows 32*(k%2)..+32);
    # single strided pass does the interleave and the fp16->f32 cast together
    out = np.empty((B, F, H, W), np.float32)
    out.reshape(B, F, 2, ROWS, 64)[:] = res.reshape(B, 2, F, ROWS, 64).transpose(
        0, 2, 1, 3, 4
    )
    return out


# revision 33
# speedup vs baseline: 1.0786x; 1.0786x over previous
"""KAConv (rational-function conv) Trainium2 Bass kernel, 8-core SPMD.

Math per output (b,f,h,w):
  out = sum_{c,p} P_fcp(x_win) / (1 + |Q_fcp(x_win)|)
with P = deg-5 poly (6 coeffs), Q = powers 1..4 (4 coeffs), win = 3x3 offsets.

Strategy (all shapes hardcoded for B=4,C=16,F=16,H=W=64,K=3):
- Shard spatial: core k handles batch k//2, H-rows 32*(k%2) .. +32  (2048 pts).
- On device, build power tensors PW [48, 2244] = rows (k*8 + c_local) holding
  x^k for 8 channels over the 34x66 zero-padded slice; two tensors for the
  two channel octets. x ships as fp16 (half the bytes; rel err ~4e-3, well
  inside the 2e-2 gate) and is upconverted on device.
- Coefficients (+ fold selector) ship compact and SHARDED: 11 rows/core of
  an [88, 288] fp16 block, AllGathered on device, upconverted to f32 on
  SBUF, then scattered into the block-diagonal lhsT layout ([48, 18*128])
  via 80 single-partition DMAs after a memset. This replaces 8.8 MB of
  host->device traffic (93%-zero block-diag tensors, replicated x8) with
  ~0.62 MB total. Total wire error (x fp16 + coef fp16 + out fp16) is
  5.6e-3 vs the 2e-2 gate.
- P and Q for 8 channels x 16 filters at once via one K=48, M=128, N=512
  block-diagonal matmul on TensorE per (octet, kernel-offset, 512-pt chunk).
- Consumers (full 128-lane ops): |q| -> Abs (ACT), ln(1+|q|) (ACT, bias=1),
  r=exp(-l) (ACT), t = P*r (DVE TT), then sum over channels via a selector
  matmul E.T @ t accumulated into PSUM across all 72 units.
- The per-core fp16 result is AllGathered on device into a replicated
  [128, 2048] output, so the host fetch is one round-trip from one device.
- Dispatch: the shard_map executable is built and AOT-compiled once and
  cached (run_bass_kernel_spmd re-traces every call, ~185 ms). The donated
  output buffer is recycled from the previous call's device-resident
  result, so no zero-buffer upload per call. First kernel() call runs a
  few warmups so subsequent calls hit the steady-state fast path (~68 ms,
  dominated by the axon-tunnel execute round-trip).
"""

import numpy as np

import jax
from jax.sharding import Mesh, PartitionSpec
from jax.experimental.shard_map import shard_map

import concourse.bass as bass
import concourse.bacc as bacc
import concourse.tile as tile
import concourse.mybir as mybir
from concourse.bass2jax import (
    _bass_exec_p,
    partition_id_tensor,
    install_neuronx_cc_hook,
)

F32 = mybir.dt.float32
F16 = mybir.dt.float16
AF = mybir.ActivationFunctionType

B, C, F, H, W = 4, 16, 16, 64, 64
PH, PW_ = 34, 66          # padded slice dims per core (32+2 rows, 64+2 cols)
NPIX = PH * PW_           # 2244
ROWS, CHUNK = 32, 512     # output rows per core, free-dim chunk (8 rows x 64)
NCH = 4                   # chunks per core (4 x 512 = 2048 pts)
DEG_P, DEG_Q, KK = 6, 4, 9
NCORES = 8

_cache = {}


def _build_program():
    nc = bacc.Bacc("TRN2", target_bir_lowering=False, debug=False, num_devices=NCORES)

    xh = nc.dram_tensor("xh", [C, NPIX], F16, kind="ExternalInput").ap()
    # compact coefficients arrive fp16, sharded 11 rows/core, AllGathered on
    # device: rows 0..47 A, 48..79 Bc, 80..87 the fold selector (16x16 per row)
    cshard = nc.dram_tensor("cshard", [11, 2 * KK * 16], F16, kind="ExternalInput").ap()
    cstage = nc.dram_tensor("cstage", [11, 2 * KK * 16], F16, kind="Internal").ap()
    cfull = nc.dram_tensor(
        "cfull", [88, 2 * KK * 16], F16, kind="Internal", addr_space="Shared"
    ).ap()
    # per-core result is int8-quantized (per-row abs-max scale, f32 scale
    # embedded in the last 4 bytes of each row) and AllGathered on device so
    # the host fetches the full output from a single device in ~262 KB
    I8 = mybir.dt.int8
    OCOLS = ROWS * 64 + 4
    ostage = nc.dram_tensor("ostage", [16, OCOLS], I8, kind="Internal").ap()
    ofull = nc.dram_tensor(
        "ofull", [128, OCOLS], I8, kind="Internal", addr_space="Shared"
    ).ap()
    out = nc.dram_tensor("out", [128, OCOLS], I8, kind="ExternalOutput").ap()

    with tile.TileContext(nc) as tc:
        with (
            tc.tile_pool(name="persist", bufs=1) as pp_persist,
            tc.tile_pool(name="work", bufs=4) as pw_work,
            tc.tile_pool(name="psum", bufs=2, space=bass.MemorySpace.PSUM) as pp_psum,
            tc.tile_pool(name="psacc", bufs=1, space=bass.MemorySpace.PSUM) as pp_acc,
        ):
            # ---- coefficient AllGather kicked off first (overlaps x work) ----
            nc.sync.dma_start(cstage[:], cshard[:])
            nc.gpsimd.collective_compute(
                "AllGather",
                mybir.AluOpType.bypass,
                replica_groups=[list(range(NCORES))],
                ins=[cstage],
                outs=[cfull],
            )

            # ---- x: fp16 in, upconvert to f32 rows 8..16 of both PW tensors ----
            xhs = pp_persist.tile([C, NPIX], F16, tag="xhs")
            nc.sync.dma_start(xhs[:], xh[:])

            x1 = pp_persist.tile([C, NPIX], F32, tag="x1")
            nc.scalar.activation(x1[:], xhs[:], AF.Copy)
            x2 = pp_persist.tile([C, NPIX], F32, tag="x2")
            nc.vector.tensor_mul(x2[:], x1[:], x1[:])
            x3 = pp_persist.tile([C, NPIX], F32, tag="x3")
            nc.vector.tensor_mul(x3[:], x2[:], x1[:])
            x4 = pp_persist.tile([C, NPIX], F32, tag="x4")
            nc.vector.tensor_mul(x4[:], x2[:], x2[:])
            x5 = pp_persist.tile([C, NPIX], F32, tag="x5")
            nc.vector.tensor_mul(x5[:], x2[:], x3[:])

            # PW tensors: rows 8k + cl hold x_{oct*8+cl}^k
            pwa = pp_persist.tile([48, NPIX], F32, tag="pwa")
            pwb = pp_persist.tile([48, NPIX], F32, tag="pwb")
            for oct_, pwt in ((0, pwa), (1, pwb)):
                nc.vector.memset(pwt[0:8, :], 1.0)
                for k, xk in ((1, x1), (2, x2), (3, x3), (4, x4), (5, x5)):
                    nc.sync.dma_start(
                        pwt[8 * k : 8 * k + 8, :], xk[8 * oct_ : 8 * oct_ + 8, :]
                    )

            # ---- block-diag lhsT tiles scattered from compact coef ----
            # gathered fp16 coef is upconverted once on SBUF, then scattered
            cf16 = pp_persist.tile([88, 2 * KK * 16], F16, tag="cf16")
            nc.sync.dma_start(cf16[:], cfull[:])
            cf32 = pp_persist.tile([88, 2 * KK * 16], F32, tag="cf32")
            nc.scalar.activation(cf32[:], cf16[:], AF.Copy)

            cps = pp_persist.tile([48, 2 * KK * 128], F32, tag="cps")
            cqs = pp_persist.tile([48, 2 * KK * 128], F32, tag="cqs")
            nc.vector.memset(cps[:], 0.0)
            nc.vector.memset(cqs[:], 0.0)
            # one DMA per compact row: row r of A lands in cps partition r at
            # column offset 16*(r%8) of each 128-wide block; B rows shift down
            # one power-octet (8 partitions). Single-partition APs keep the
            # race tracker exact (partition-strided writes are over-approximated).
            cview = cps[:].rearrange("r (u w) -> r u w", w=128)
            qview = cqs[:].rearrange("r (u w) -> r u w", w=128)
            csrc = cf32[:].rearrange("r (u v) -> r u v", v=16)
            for r in range(48):
                cl = r % 8
                nc.sync.dma_start(
                    cview[r : r + 1, :, 16 * cl : 16 * cl + 16], csrc[r : r + 1]
                )
            for r in range(32):
                cl = r % 8
                nc.sync.dma_start(
                    qview[r + 8 : r + 9, :, 16 * cl : 16 * cl + 16],
                    csrc[48 + r : 49 + r],
                )

            ef = pp_persist.tile([128, 16], F32, tag="ef")
            nc.sync.dma_start(
                ef[:],
                cf32[80:88, 0 : 16 * 16].rearrange("e (q v) -> e q v", v=16),
            )

            acc = pp_acc.tile([16, NCH * CHUNK], F32, tag="acc")
            osb = pp_persist.tile([16, NCH * CHUNK + 4], mybir.dt.int8, tag="osb")

            # ---- main loop ----
            # fold-MM for unit u is emitted after unit u+1's P/Q matmuls so
            # the in-order PE queue never stalls behind unit u's ACT/DVE chain.
            pending = None  # (tt, ch, first)
            folds_done = [0] * NCH

            def emit_fold(pend):
                tt_, ch_, first_ = pend
                folds_done[ch_] += 1
                nc.tensor.matmul(
                    acc[:, ch_ * CHUNK : (ch_ + 1) * CHUNK],
                    ef[:],
                    tt_[:],
                    start=first_,
                    stop=folds_done[ch_] == 2 * KK,
                    skip_group_check=True,
                )

            for oct_ in range(2):
                pwt = pwa if oct_ == 0 else pwb
                pw3 = pwt[:].rearrange("p (h w) -> p h w", w=PW_)
                for p in range(KK):
                    di, dj = p // 3, p % 3
                    lhsP = cps[:, (oct_ * KK + p) * 128 : (oct_ * KK + p) * 128 + 128]
                    lhsQ = cqs[:, (oct_ * KK + p) * 128 : (oct_ * KK + p) * 128 + 128]
                    for ch in range(NCH):
                        r0 = ch * 8 + di
                        rhs = pw3[:, r0 : r0 + 8, dj : dj + 64]
                        pp = pp_psum.tile([128, CHUNK], F32, tag="pp")
                        nc.tensor.matmul(pp[:], lhsP, rhs, start=True, stop=True)
                        qq = pp_psum.tile([128, CHUNK], F32, tag="qq")
                        nc.tensor.matmul(qq[:], lhsQ, rhs, start=True, stop=True)
                        if pending is not None:
                            emit_fold(pending)

                        dd = pw_work.tile([128, CHUNK], F32, tag="dd")
                        nc.scalar.activation(dd[:], qq[:], AF.Abs)
                        ll = pw_work.tile([128, CHUNK], F32, tag="ll")
                        nc.scalar.activation(ll[:], dd[:], AF.Ln, bias=1.0)
                        rr = pw_work.tile([128, CHUNK], F32, tag="rr")
                        nc.scalar.activation(rr[:], ll[:], AF.Exp, scale=-1.0)
                        tt = pw_work.tile([128, CHUNK], F32, tag="tt")
                        nc.vector.tensor_mul(tt[:], pp[:], rr[:])
                        pending = (tt, ch, folds_done[ch] == 0)
            emit_fold(pending)

            # int8 quantization: rmax = per-row max|acc|, q = acc * 127/rmax
            rmax = pp_persist.tile([16, 1], F32, tag="rmax")
            nc.vector.tensor_reduce(
                rmax[:],
                acc[:],
                axis=mybir.AxisListType.X,
                op=mybir.AluOpType.max,
                apply_absolute_value=True,
            )
            rscl = pp_persist.tile([16, 1], F32, tag="rscl")
            nc.vector.reciprocal(rscl[:], rmax[:])
            nc.vector.tensor_scalar_mul(rscl[:], rscl[:], 127.0)
            nc.scalar.activation(
                osb[:, 0 : NCH * CHUNK], acc[:], AF.Copy, scale=rscl[:]
            )
            nc.sync.dma_start(osb[:, NCH * CHUNK :].bitcast(F32), rmax[:])
            nc.sync.dma_start(ostage[:], osb[:])
            nc.gpsimd.collective_compute(
                "AllGather",
                mybir.AluOpType.bypass,
                replica_groups=[list(range(NCORES))],
                ins=[ostage],
                outs=[ofull],
            )
            nc.sync.dma_start(out[:], ofull[:])

    nc.compile()
    return nc


def _make_dispatch(nc):
    """Build the cached jitted shard_map executable (bass_exec custom call)."""
    install_neuronx_cc_hook()
    partition_name = nc.partition_id_tensor.name if nc.partition_id_tensor else None
    in_names, out_names, out_avals = [], [], []
    for alloc in nc.m.functions[0].allocations:
        if not isinstance(alloc, mybir.MemoryLocationSet):
            continue
        name = alloc.memorylocations[0].name
        if alloc.kind == "ExternalInput":
            if name != partition_name:
                in_names.append(name)
        elif alloc.kind == "ExternalOutput":
            out_names.append(name)
            out_avals.append(
                jax.core.ShapedArray(
                    tuple(alloc.tensor_shape), mybir.dt.np(alloc.dtype)
                )
            )
    n_params = len(in_names)
    in_names_full = in_names + out_names
    if partition_name is not None:
        in_names_full.append(partition_name)

    def _body(*args):
        operands = list(args)
        if partition_name is not None:
            operands.append(partition_id_tensor())
        outs = _bass_exec_p.bind(
            *operands,
            out_avals=tuple(out_avals),
            in_names=tuple(in_names_full),
            out_names=tuple(out_names),
            lowering_input_output_aliases=(),
            sim_require_finite=True,
            sim_require_nnan=True,
            nc=nc,
        )
        return tuple(outs)

    donate = tuple(range(n_params, n_params + len(out_names)))
    devices = jax.devices()[:NCORES]
    mesh = Mesh(np.asarray(devices), ("core",))
    # inputs row-sharded; the (donated) output is replicated — every core
    # holds the full AllGathered result, so the host fetch reads one shard
    fn = jax.jit(
        shard_map(
            _body,
            mesh=mesh,
            in_specs=(PartitionSpec("core"),) * n_params
            + (PartitionSpec(),) * len(out_names),
            out_specs=(PartitionSpec(),) * len(out_names),
            check_rep=False,
        ),
        donate_argnums=donate,
        keep_unused=True,
    )
    return fn, in_names, out_names


def _prep_inputs(x, A, Bc):
    """Host-side marshalling: fp16 padded slices + compact coefficient tiles."""
    # core k = (batch k//2, half k%2) holds padded rows 32*half..+34. The
    # zero borders of the staging buffer are static, so it is allocated once
    # and only the data interior is rewritten per call.
    xh4 = _cache.get("xh_buf")
    if xh4 is None:
        xh4 = _cache["xh_buf"] = np.zeros((NCORES, C, PH, PW_), np.float16)
    x16 = x.astype(np.float16)
    for k in range(NCORES):
        bk, half = k // 2, k % 2
        if half == 0:
            xh4[k, :, 1:34, 1:65] = x16[bk, :, 0:33, :]
        else:
            xh4[k, :, 0:33, 1:65] = x16[bk, :, 31:64, :]
    xh = xh4.reshape(NCORES * C, NPIX)

    # compact coefficients: rows 0..47 = A (8k+cl), rows 48..79 = Bc (8j+cl);
    # cols = (oct*9 + p)*16 + f
    ca = (
        A.transpose(3, 1, 2, 0)            # [k, c, p, f]
        .reshape(DEG_P, 2, 8, KK, 16)      # [k, oct, cl, p, f]
        .transpose(0, 2, 1, 3, 4)          # [k, cl, oct, p, f]
        .reshape(48, 2 * KK * 16)
    )
    cb = (
        Bc.transpose(3, 1, 2, 0)
        .reshape(DEG_Q, 2, 8, KK, 16)
        .transpose(0, 2, 1, 3, 4)
        .reshape(32, 2 * KK * 16)
    )
    ef = np.zeros((128, 16), np.float32)
    ef[np.arange(128), np.arange(128) % 16] = 1.0
    efpack = np.zeros((8, 2 * KK * 16), np.float32)
    efpack[:, : 16 * 16] = ef.reshape(8, 256)

    # global [88, 288] fp16 (0/1 selector rows are exact in fp16):
    # jax row-shards it 11 rows/core; device AllGather rebuilds
    cshard_all = np.concatenate([ca, cb, efpack], axis=0).astype(np.float16)
    return xh, cshard_all


def _run(xh, cshard_all):
    fn = _cache.get("fn_compiled") or _cache["fn"]
    prev_out = _cache.get("prev_out")
    if prev_out is None:
        prev_out = np.zeros((NCORES * 16, ROWS * 64 + 4), np.int8)  # [128, 2052]
    try:
        (out,) = fn(xh, cshard_all, prev_out)
    except Exception:
        if "fn_compiled" not in _cache:
            raise
        # AOT path rejected the inputs; fall back to the traced jit
        _cache.pop("fn_compiled")
        (out,) = _cache["fn"](xh, cshard_all, prev_out)
    try:
        # queue the D2H copy behind the execute now rather than when
        # np.asarray blocks — starts the fetch one round-trip earlier
        out.copy_to_host_async()
    except Exception:
        pass
    res = np.asarray(out)
    _cache["prev_out"] = out
    return res


def kernel(x, A, Bc):
    x = np.asarray(x, np.float32)
    A = np.asarray(A, np.float32)
    Bc = np.asarray(Bc, np.float32)
    xh, cshard_all = _prep_inputs(x, A, Bc)
    if "fn" not in _cache:
        nc = _build_program()
        fn, in_names, out_names = _make_dispatch(nc)
        assert in_names == ["xh", "cshard"] and out_names == ["out"]
        _cache["nc"] = nc
        _cache["fn"] = fn
        # AOT-compile to skip per-call jit dispatch bookkeeping (~3 ms)
        try:
            _cache["fn_compiled"] = fn.lower(
                xh,
                cshard_all,
                np.zeros((NCORES * 16, ROWS * 64 + 4), np.int8),
            ).compile()
        except Exception:
            pass
        # warm the axon/PJRT path (through the same executable later calls
        # use) so they hit steady state
        for _ in range(5):
            res = _run(xh, cshard_all)
    res = _run(xh, cshard_all)
    # res: [8*16, 2052] int8; cols 0..2048 = q, last 4 bytes = f32 row scale.
    # core k = (batch k//2, rows 32*(k%2)..+32); dequantize + interleave.
    q = res[:, : ROWS * 64]
    rmax = res[:, ROWS * 64 :].copy().view(np.float32)  # [128, 1]
    deq = q.astype(np.float32) * (rmax * (1.0 / 127.0))
    out = np.empty((B, F, H, W), np.float32)
    out.reshape(B, F, 2, ROWS, 64)[:] = deq.reshape(B, 2, F, ROWS, 64).transpose(
        0, 2, 1, 3, 4
    )
    return out


# revision 35
# speedup vs baseline: 1.0986x; 1.0185x over previous
"""KAConv (rational-function conv) Trainium2 Bass kernel, 8-core SPMD.

Math per output (b,f,h,w):
  out = sum_{c,p} P_fcp(x_win) / (1 + |Q_fcp(x_win)|)
with P = deg-5 poly (6 coeffs), Q = powers 1..4 (4 coeffs), win = 3x3 offsets.

Strategy (all shapes hardcoded for B=4,C=16,F=16,H=W=64,K=3):
- Shard spatial: core k handles batch k//2, H-rows 32*(k%2) .. +32  (2048 pts).
- On device, build power tensors PW [48, 2244] = rows (k*8 + c_local) holding
  x^k for 8 channels over the 34x66 zero-padded slice; two tensors for the
  two channel octets. x ships as fp16 (half the bytes; rel err ~4e-3, well
  inside the 2e-2 gate) and is upconverted on device.
- Coefficients (+ fold selector) ship compact and SHARDED: 11 rows/core of
  an [88, 288] fp16 block, AllGathered on device, upconverted to f32 on
  SBUF, then scattered into the block-diagonal lhsT layout ([48, 18*128])
  via 80 single-partition DMAs after a memset. This replaces 8.8 MB of
  host->device traffic (93%-zero block-diag tensors, replicated x8) with
  ~0.62 MB total. Total wire error (x fp16 + coef fp16 + out fp16) is
  5.6e-3 vs the 2e-2 gate.
- P and Q for 8 channels x 16 filters at once via one K=48, M=128, N=512
  block-diagonal matmul on TensorE per (octet, kernel-offset, 512-pt chunk).
- Consumers (full 128-lane ops): |q| -> Abs (ACT), ln(1+|q|) (ACT, bias=1),
  r=exp(-l) (ACT), t = P*r (DVE TT), then sum over channels via a selector
  matmul E.T @ t accumulated into PSUM across all 72 units.
- The per-core result is int8-quantized (per-row abs-max scale from a DVE
  abs-max reduce; the f32 scale rides in the last 4 bytes of each row) and
  AllGathered on device into a replicated [128, 2052] output, so the host
  fetch is one ~262 KB round-trip from one device. Measured end-to-end
  error 5.5e-3 vs the 2e-2 gate.
- Dispatch: the shard_map executable is built and AOT-compiled once and
  cached (run_bass_kernel_spmd re-traces every call, ~185 ms). The donated
  output buffer is recycled from the previous call's device-resident
  result, so no zero-buffer upload per call. First kernel() call runs a
  few warmups so subsequent calls hit the steady-state fast path (~68 ms,
  dominated by the axon-tunnel execute round-trip).
"""

import numpy as np

import jax
from jax.sharding import Mesh, PartitionSpec
from jax.experimental.shard_map import shard_map

import concourse.bass as bass
import concourse.bacc as bacc
import concourse.tile as tile
import concourse.mybir as mybir
from concourse.bass2jax import (
    _bass_exec_p,
    partition_id_tensor,
    install_neuronx_cc_hook,
)

F32 = mybir.dt.float32
F16 = mybir.dt.float16
AF = mybir.ActivationFunctionType

B, C, F, H, W = 4, 16, 16, 64, 64
PH, PW_ = 34, 66          # padded slice dims per core (32+2 rows, 64+2 cols)
NPIX = PH * PW_           # 2244
ROWS, CHUNK = 32, 512     # output rows per core, free-dim chunk (8 rows x 64)
NCH = 4                   # chunks per core (4 x 512 = 2048 pts)
DEG_P, DEG_Q, KK = 6, 4, 9
NCORES = 8

_cache = {}


def _build_program():
    nc = bacc.Bacc("TRN2", target_bir_lowering=False, debug=False, num_devices=NCORES)

    xh = nc.dram_tensor("xh", [C, NPIX], F16, kind="ExternalInput").ap()
    # compact coefficients arrive fp16, sharded 11 rows/core, AllGathered on
    # device: rows 0..47 A, 48..79 Bc, 80..87 the fold selector (16x16 per row)
    cshard = nc.dram_tensor("cshard", [11, 2 * KK * 16], F16, kind="ExternalInput").ap()
    cstage = nc.dram_tensor("cstage", [11, 2 * KK * 16], F16, kind="Internal").ap()
    cfull = nc.dram_tensor(
        "cfull", [88, 2 * KK * 16], F16, kind="Internal", addr_space="Shared"
    ).ap()
    # per-core result is int8-quantized (per-row abs-max scale, f32 scale
    # embedded in the last 4 bytes of each row) and AllGathered on device so
    # the host fetches the full output from a single device in ~262 KB
    I8 = mybir.dt.int8
    OCOLS = ROWS * 64 + 4
    ostage = nc.dram_tensor("ostage", [16, OCOLS], I8, kind="Internal").ap()
    ofull = nc.dram_tensor(
        "ofull", [128, OCOLS], I8, kind="Internal", addr_space="Shared"
    ).ap()
    out = nc.dram_tensor("out", [128, OCOLS], I8, kind="ExternalOutput").ap()

    with tile.TileContext(nc) as tc:
        with (
            tc.tile_pool(name="persist", bufs=1) as pp_persist,
            tc.tile_pool(name="work", bufs=4) as pw_work,
            tc.tile_pool(name="psum", bufs=2, space=bass.MemorySpace.PSUM) as pp_psum,
            tc.tile_pool(name="psacc", bufs=1, space=bass.MemorySpace.PSUM) as pp_acc,
        ):
            # ---- coefficient AllGather kicked off first (overlaps x work) ----
            nc.sync.dma_start(cstage[:], cshard[:])
            nc.gpsimd.collective_compute(
                "AllGather",
                mybir.AluOpType.bypass,
                replica_groups=[list(range(NCORES))],
                ins=[cstage],
                outs=[cfull],
            )

            # ---- x: fp16 in, upconvert to f32 rows 8..16 of both PW tensors ----
            xhs = pp_persist.tile([C, NPIX], F16, tag="xhs")
            nc.sync.dma_start(xhs[:], xh[:])

            x1 = pp_persist.tile([C, NPIX], F32, tag="x1")
            nc.scalar.activation(x1[:], xhs[:], AF.Copy)
            x2 = pp_persist.tile([C, NPIX], F32, tag="x2")
            nc.vector.tensor_mul(x2[:], x1[:], x1[:])
            x3 = pp_persist.tile([C, NPIX], F32, tag="x3")
            nc.vector.tensor_mul(x3[:], x2[:], x1[:])
            x4 = pp_persist.tile([C, NPIX], F32, tag="x4")
            nc.vector.tensor_mul(x4[:], x2[:], x2[:])
            x5 = pp_persist.tile([C, NPIX], F32, tag="x5")
            nc.vector.tensor_mul(x5[:], x2[:], x3[:])

            # PW tensors: rows 8k + cl hold x_{oct*8+cl}^k
            pwa = pp_persist.tile([48, NPIX], F32, tag="pwa")
            pwb = pp_persist.tile([48, NPIX], F32, tag="pwb")
            for oct_, pwt in ((0, pwa), (1, pwb)):
                nc.vector.memset(pwt[0:8, :], 1.0)
                for k, xk in ((1, x1), (2, x2), (3, x3), (4, x4), (5, x5)):
                    nc.sync.dma_start(
                        pwt[8 * k : 8 * k + 8, :], xk[8 * oct_ : 8 * oct_ + 8, :]
                    )

            # ---- block-diag lhsT tiles scattered from compact coef ----
            # gathered fp16 coef is upconverted once on SBUF, then scattered
            cf16 = pp_persist.tile([88, 2 * KK * 16], F16, tag="cf16")
            nc.sync.dma_start(cf16[:], cfull[:])
            cf32 = pp_persist.tile([88, 2 * KK * 16], F32, tag="cf32")
            nc.scalar.activation(cf32[:], cf16[:], AF.Copy)

            cps = pp_persist.tile([48, 2 * KK * 128], F32, tag="cps")
            cqs = pp_persist.tile([48, 2 * KK * 128], F32, tag="cqs")
            nc.vector.memset(cps[:], 0.0)
            nc.vector.memset(cqs[:], 0.0)
            # one DMA per compact row: row r of A lands in cps partition r at
            # column offset 16*(r%8) of each 128-wide block; B rows shift down
            # one power-octet (8 partitions). Single-partition APs keep the
            # race tracker exact (partition-strided writes are over-approximated).
            cview = cps[:].rearrange("r (u w) -> r u w", w=128)
            qview = cqs[:].rearrange("r (u w) -> r u w", w=128)
            csrc = cf32[:].rearrange("r (u v) -> r u v", v=16)
            for r in range(48):
                cl = r % 8
                nc.sync.dma_start(
                    cview[r : r + 1, :, 16 * cl : 16 * cl + 16], csrc[r : r + 1]
                )
            for r in range(32):
                cl = r % 8
                nc.sync.dma_start(
                    qview[r + 8 : r + 9, :, 16 * cl : 16 * cl + 16],
                    csrc[48 + r : 49 + r],
                )

            ef = pp_persist.tile([128, 16], F32, tag="ef")
            nc.sync.dma_start(
                ef[:],
                cf32[80:88, 0 : 16 * 16].rearrange("e (q v) -> e q v", v=16),
            )

            acc = pp_acc.tile([16, NCH * CHUNK], F32, tag="acc")
            osb = pp_persist.tile([16, NCH * CHUNK + 4], mybir.dt.int8, tag="osb")

            # ---- main loop ----
            # fold-MM for unit u is emitted after unit u+1's P/Q matmuls so
            # the in-order PE queue never stalls behind unit u's ACT/DVE chain.
            pending = None  # (tt, ch, first)
            folds_done = [0] * NCH

            def emit_fold(pend):
                tt_, ch_, first_ = pend
                folds_done[ch_] += 1
                nc.tensor.matmul(
                    acc[:, ch_ * CHUNK : (ch_ + 1) * CHUNK],
                    ef[:],
                    tt_[:],
                    start=first_,
                    stop=folds_done[ch_] == 2 * KK,
                    skip_group_check=True,
                )

            for oct_ in range(2):
                pwt = pwa if oct_ == 0 else pwb
                pw3 = pwt[:].rearrange("p (h w) -> p h w", w=PW_)
                for p in range(KK):
                    di, dj = p // 3, p % 3
                    lhsP = cps[:, (oct_ * KK + p) * 128 : (oct_ * KK + p) * 128 + 128]
                    lhsQ = cqs[:, (oct_ * KK + p) * 128 : (oct_ * KK + p) * 128 + 128]
                    for ch in range(NCH):
                        r0 = ch * 8 + di
                        rhs = pw3[:, r0 : r0 + 8, dj : dj + 64]
                        pp = pp_psum.tile([128, CHUNK], F32, tag="pp")
                        nc.tensor.matmul(pp[:], lhsP, rhs, start=True, stop=True)
                        qq = pp_psum.tile([128, CHUNK], F32, tag="qq")
                        nc.tensor.matmul(qq[:], lhsQ, rhs, start=True, stop=True)
                        if pending is not None:
                            emit_fold(pending)

                        dd = pw_work.tile([128, CHUNK], F32, tag="dd")
                        nc.scalar.activation(dd[:], qq[:], AF.Abs)
                        ll = pw_work.tile([128, CHUNK], F32, tag="ll")
                        nc.scalar.activation(ll[:], dd[:], AF.Ln, bias=1.0)
                        rr = pw_work.tile([128, CHUNK], F32, tag="rr")
                        nc.scalar.activation(rr[:], ll[:], AF.Exp, scale=-1.0)
                        tt = pw_work.tile([128, CHUNK], F32, tag="tt")
                        nc.vector.tensor_mul(tt[:], pp[:], rr[:])
                        pending = (tt, ch, folds_done[ch] == 0)
            emit_fold(pending)

            # int8 quantization: rmax = per-row max|acc|, q = acc * 127/rmax
            rmax = pp_persist.tile([16, 1], F32, tag="rmax")
            nc.vector.tensor_reduce(
                rmax[:],
                acc[:],
                axis=mybir.AxisListType.X,
                op=mybir.AluOpType.max,
                apply_absolute_value=True,
            )
            rscl = pp_persist.tile([16, 1], F32, tag="rscl")
            nc.vector.reciprocal(rscl[:], rmax[:])
            nc.vector.tensor_scalar_mul(rscl[:], rscl[:], 127.0)
            nc.scalar.activation(
                osb[:, 0 : NCH * CHUNK], acc[:], AF.Copy, scale=rscl[:]
            )
            nc.sync.dma_start(osb[:, NCH * CHUNK :].bitcast(F32), rmax[:])
            nc.sync.dma_start(ostage[:], osb[:])
            nc.gpsimd.collective_compute(
                "AllGather",
                mybir.AluOpType.bypass,
                replica_groups=[list(range(NCORES))],
                ins=[ostage],
                outs=[ofull],
            )
            nc.sync.dma_start(out[:], ofull[:])

    nc.compile()
    return nc


def _make_dispatch(nc):
    """Build the cached jitted shard_map executable (bass_exec custom call)."""
    install_neuronx_cc_hook()
    partition_name = nc.partition_id_tensor.name if nc.partition_id_tensor else None
    in_names, out_names, out_avals = [], [], []
    for alloc in nc.m.functions[0].allocations:
        if not isinstance(alloc, mybir.MemoryLocationSet):
            continue
        name = alloc.memorylocations[0].name
        if alloc.kind == "ExternalInput":
            if name != partition_name:
                in_names.append(name)
        elif alloc.kind == "ExternalOutput":
            out_names.append(name)
            out_avals.append(
                jax.core.ShapedArray(
                    tuple(alloc.tensor_shape), mybir.dt.np(alloc.dtype)
                )
            )
    n_params = len(in_names)
    in_names_full = in_names + out_names
    if partition_name is not None:
        in_names_full.append(partition_name)

    def _body(*args):
        operands = list(args)
        if partition_name is not None:
            operands.append(partition_id_tensor())
        outs = _bass_exec_p.bind(
            *operands,
            out_avals=tuple(out_avals),
            in_names=tuple(in_names_full),
            out_names=tuple(out_names),
            lowering_input_output_aliases=(),
            sim_require_finite=True,
            sim_require_nnan=True,
            nc=nc,
        )
        return tuple(outs)

    donate = tuple(range(n_params, n_params + len(out_names)))
    devices = jax.devices()[:NCORES]
    mesh = Mesh(np.asarray(devices), ("core",))
    # inputs row-sharded; the (donated) output is replicated — every core
    # holds the full AllGathered result, so the host fetch reads one shard
    fn = jax.jit(
        shard_map(
            _body,
            mesh=mesh,
            in_specs=(PartitionSpec("core"),) * n_params
            + (PartitionSpec(),) * len(out_names),
            out_specs=(PartitionSpec(),) * len(out_names),
            check_rep=False,
        ),
        donate_argnums=donate,
        keep_unused=True,
    )
    return fn, in_names, out_names


def _prep_inputs(x, A, Bc):
    """Host-side marshalling: fp16 padded slices + compact coefficient tiles."""
    # core k = (batch k//2, half k%2) holds padded rows 32*half..+34. The
    # zero borders of the staging buffer are static, so it is allocated once
    # and only the data interior is rewritten per call.
    xh4 = _cache.get("xh_buf")
    if xh4 is None:
        xh4 = _cache["xh_buf"] = np.zeros((NCORES, C, PH, PW_), np.float16)
    x16 = x.astype(np.float16)
    for k in range(NCORES):
        bk, half = k // 2, k % 2
        if half == 0:
            xh4[k, :, 1:34, 1:65] = x16[bk, :, 0:33, :]
        else:
            xh4[k, :, 0:33, 1:65] = x16[bk, :, 31:64, :]
    xh = xh4.reshape(NCORES * C, NPIX)

    # compact coefficients: rows 0..47 = A (8k+cl), rows 48..79 = Bc (8j+cl);
    # cols = (oct*9 + p)*16 + f
    ca = (
        A.transpose(3, 1, 2, 0)            # [k, c, p, f]
        .reshape(DEG_P, 2, 8, KK, 16)      # [k, oct, cl, p, f]
        .transpose(0, 2, 1, 3, 4)          # [k, cl, oct, p, f]
        .reshape(48, 2 * KK * 16)
    )
    cb = (
        Bc.transpose(3, 1, 2, 0)
        .reshape(DEG_Q, 2, 8, KK, 16)
        .transpose(0, 2, 1, 3, 4)
        .reshape(32, 2 * KK * 16)
    )
    ef = np.zeros((128, 16), np.float32)
    ef[np.arange(128), np.arange(128) % 16] = 1.0
    efpack = np.zeros((8, 2 * KK * 16), np.float32)
    efpack[:, : 16 * 16] = ef.reshape(8, 256)

    # global [88, 288] fp16 (0/1 selector rows are exact in fp16):
    # jax row-shards it 11 rows/core; device AllGather rebuilds
    cshard_all = np.concatenate([ca, cb, efpack], axis=0).astype(np.float16)
    return xh, cshard_all


def _run(xh, cshard_all):
    fn = _cache.get("fn_compiled") or _cache["fn"]
    prev_out = _cache.get("prev_out")
    if prev_out is None:
        prev_out = np.zeros((NCORES * 16, ROWS * 64 + 4), np.int8)  # [128, 2052]
    try:
        (out,) = fn(xh, cshard_all, prev_out)
    except Exception:
        if "fn_compiled" not in _cache:
            raise
        # AOT path rejected the inputs; fall back to the traced jit
        _cache.pop("fn_compiled")
        (out,) = _cache["fn"](xh, cshard_all, prev_out)
    try:
        # queue the D2H copy behind the execute now rather than when
        # np.asarray blocks — starts the fetch one round-trip earlier
        out.copy_to_host_async()
    except Exception:
        pass
    res = np.asarray(out)
    _cache["prev_out"] = out
    return res


def kernel(x, A, Bc):
    x = np.asarray(x, np.float32)
    A = np.asarray(A, np.float32)
    Bc = np.asarray(Bc, np.float32)
    xh, cshard_all = _prep_inputs(x, A, Bc)
    if "fn" not in _cache:
        nc = _build_program()
        fn, in_names, out_names = _make_dispatch(nc)
        assert in_names == ["xh", "cshard"] and out_names == ["out"]
        _cache["nc"] = nc
        _cache["fn"] = fn
        # AOT-compile to skip per-call jit dispatch bookkeeping (~3 ms)
        try:
            _cache["fn_compiled"] = fn.lower(
                xh,
                cshard_all,
                np.zeros((NCORES * 16, ROWS * 64 + 4), np.int8),
            ).compile()
        except Exception:
            pass
        # warm the axon/PJRT path (through the same executable later calls
        # use) so they hit steady state
        for _ in range(5):
            res = _run(xh, cshard_all)
    res = _run(xh, cshard_all)
    # res: [8*16, 2052] int8; cols 0..2048 = q, last 4 bytes = f32 row scale.
    # core k = (batch k//2, rows 32*(k%2)..+32); dequantize + interleave.
    q = res[:, : ROWS * 64]
    rmax = res[:, ROWS * 64 :].copy().view(np.float32)  # [128, 1]
    out = np.empty((B, F, H, W), np.float32)
    # one pass: int8->f32 cast, per-row scale, and half-interleave together
    np.multiply(
        q.reshape(B, 2, F, ROWS, 64).transpose(0, 2, 1, 3, 4),
        (rmax * (1.0 / 127.0)).reshape(B, 2, F, 1, 1).transpose(0, 2, 1, 3, 4),
        out=out.reshape(B, F, 2, ROWS, 64),
    )
    return out
